# revision 16
# baseline (speedup 1.0000x reference)
"""Trainium2 Bass kernel for DecouplePreAggGraphConv (GNN message passing).

out[b,j,:] = diag(adj)[j] * (x[b,j] @ W0[j])
           + sum_k offdiag(adj)[j,k] * (x[b,k] @ W1[k])
           + bias

Data-parallel over B across 8 NeuronCores. Per core, per 128-row batch
tile:
  1. one DMA load of x-tile [128, J*128]
  2. PE transposes per joint -> xT_k [n, b] (via identity matmul)
  3. per-joint GEMM  h_k = xT_k.T @ [diag_k*W0_k | W1_k]  -> PSUM [128,256]
  4. drain h to SBUF, then SBUF->SBUF DMA reshuffle into a
     (3-batch-row-group, 35-row) layout: rows = [17 h1 | 17 h0s | bias]
  5. mixing GEMM with a constant block-diagonal [105,51] stationary
     matrix (off.T / I / ones blocks) computes the adjacency mix, the
     self term and the bias add in one pass -> PSUM [51, (g,m)]
  6. drain + one strided store straight into out[b,j,m] layout.
"""

import sys

sys.path.insert(0, "/opt/trn_rl_repo")

import numpy as np

import concourse.bass as bass
import concourse.mybir as mybir
import concourse.tile as tile
from concourse import bacc
from concourse.bass_utils import run_bass_kernel_spmd

B, J, FIN, FOUT = 16384, 17, 128, 128
N_CORES = 8
TB = 128            # batch rows per tile
CJ = J * FOUT       # 2176
G3 = TB // 3        # 42 full groups of 3 rows; rows 126/127 ride as group 42
MAIN = 3 * G3       # 126
NG = G3 + 1         # 43 group slots (last one only has i=0,1 valid)
HPF = NG * FOUT     # 5504 free size of the reshuffled tile
MIXCH = 1024        # mix psum chunk (free elems)
F32 = mybir.dt.float32
BF16 = mybir.dt.bfloat16

_prog_cache: dict[int, object] = {}


def _build_program(bs: int):
    """Build the SPMD Bass program for a per-core batch shard of `bs` rows."""
    nt = bs // TB
    assert bs % TB == 0

    nc = bacc.Bacc("TRN2", target_bir_lowering=False, debug=False,
                   num_devices=N_CORES)

    xs = nc.declare_dram_parameter("xs", [bs, J, FIN], F32, isOutput=False)
    wcat = nc.declare_dram_parameter("wcat", [FIN, J, 2 * FOUT], F32,
                                     isOutput=False)
    mix3 = nc.declare_dram_parameter("mix3", [105, 51], BF16, isOutput=False)
    bias43 = nc.declare_dram_parameter("bias43", [3, HPF], BF16,
                                       isOutput=False)
    ident = nc.declare_dram_parameter("ident", [128, 128], F32, isOutput=False)
    out = nc.declare_dram_parameter("out", [bs, J, FOUT], F32, isOutput=True)

    with tile.TileContext(nc) as tc:
        with (
            tc.tile_pool(name="const", bufs=1) as cpool,
            tc.tile_pool(name="x", bufs=2) as xpool,
            tc.tile_pool(name="xt", bufs=3) as xtpool,
            tc.tile_pool(name="hsb", bufs=2) as hpool,
            tc.tile_pool(name="hp", bufs=2) as hppool,
            tc.tile_pool(name="osb", bufs=2) as opool,
            tc.tile_pool(name="tp", bufs=2, space=bass.MemorySpace.PSUM) as tpp,
            tc.tile_pool(name="hps", bufs=2, space=bass.MemorySpace.PSUM) as hpsp,
            tc.tile_pool(name="mxp", bufs=2, space=bass.MemorySpace.PSUM) as mxpp,
        ):
            # ---- constants, loaded once ----
            wcat_sb = cpool.tile([FIN, J, 2 * FOUT], F32, tag="wcat")
            nc.sync.dma_start(wcat_sb[:], wcat[:])
            mix3_sb = cpool.tile([105, 51], BF16, tag="mix3")
            nc.sync.dma_start(mix3_sb[:], mix3[:])
            id_sb = cpool.tile([128, 128], F32, tag="ident")
            nc.sync.dma_start(id_sb[:], ident[:])

            # ping-pong DRAM scratch for the reshuffle bounce; the
            # (i=2, g=42) rectangle never gets scattered into, so zero it
            # once (PE accumulates 0*garbage = NaN otherwise).
            scrs = [nc.dram_tensor(f"scr{p}", [102, HPF], BF16)
                    for p in range(2)]
            zro = cpool.tile([34, FOUT], BF16, tag="zro")
            nc.gpsimd.memset(zro[:], 0.0)
            for p in range(2):
                nc.sync.dma_start(
                    scrs[p][68:102, G3 * FOUT:], zro[:])

            for t in range(nt):
                b0 = t * TB
                # 1. load x tile
                x_t = xpool.tile([TB, J, FIN], F32, tag="x")
                nc.sync.dma_start(x_t[:], xs[b0:b0 + TB])

                # 2/3/4a. per joint: transpose, GEMM, drain (cast bf16)
                h_sb = hpool.tile([TB, 2, J, FOUT], BF16, tag="h")
                for k in range(J):
                    tp = tpp.tile([128, TB], F32, tag="tp")
                    nc.tensor.transpose(tp[:], x_t[:, k, :], id_sb[:])
                    xt = xtpool.tile([128, TB], F32, tag="xt")
                    if k % 2 == 0:
                        nc.vector.tensor_copy(xt[:], tp[:])
                    else:
                        nc.scalar.copy(xt[:], tp[:])
                    hk = hpsp.tile([TB, 2 * FOUT], F32, tag="hk")
                    nc.tensor.matmul(hk[:], xt[:], wcat_sb[:, k, :])
                    if k % 2 == 0:
                        nc.scalar.copy(h_sb[:, :, k, :], hk[:])
                    else:
                        nc.vector.tensor_copy(h_sb[:, :, k, :], hk[:])

                # 4b. reshuffle via DRAM bounce: scatter h into the group
                # layout in a DRAM scratch (rows r = i*34 + h*17 + k), then
                # read it back contiguously. DRAM APs have no partition-dim
                # restriction, so this is 3 scatter DMAs + 2 readback DMAs.
                scr = scrs[t % 2]
                sv = scr.rearrange("(i h k) (g m) -> i g h k m",
                                   i=3, h=2, k=17, g=NG, m=FOUT)
                for i in range(3):
                    ng = NG if i < 2 else G3
                    nc.sync.dma_start(sv[i, :ng], h_sb[i:TB:3])
                hp_t = hppool.tile([105, HPF], BF16, tag="hp")
                nc.sync.dma_start(hp_t[0:102, :], scr[:])
                nc.sync.dma_start(hp_t[102:105, :], bias43[:])

                # 5/6. mix GEMM chunks, drain, store
                # out[(i,j),(g,m)] = h0s[3g+i,j,m]
                #                  + sum_k off[j,k]*h1[3g+i,k,m] + bias[m]
                # (i=2, g=42) columns are garbage and never stored.
                o_sb = opool.tile([51, HPF], F32, tag="osb")
                nch = (HPF + MIXCH - 1) // MIXCH
                for c in range(nch):
                    f0 = c * MIXCH
                    fw = min(MIXCH, HPF - f0)
                    mp = mxpp.tile([51, MIXCH], F32, tag="mx")
                    for s0 in range(0, fw, 512):
                        sw = min(512, fw - s0)
                        nc.tensor.matmul(mp[:, s0:s0 + sw], mix3_sb[:],
                                         hp_t[:, f0 + s0:f0 + s0 + sw])
                    if c % 2 == 0:
                        nc.vector.tensor_copy(o_sb[:, f0:f0 + fw], mp[:, :fw])
                    else:
                        nc.scalar.copy(o_sb[:, f0:f0 + fw], mp[:, :fw])

                dst = out[b0:b0 + MAIN].rearrange("(g i) j m -> i j g m", i=3)
                nc.sync.dma_start(dst, o_sb[:, :G3 * FOUT])
                nc.sync.dma_start(out[b0 + MAIN:b0 + TB],
                                  o_sb[0:34, G3 * FOUT:])

    nc.compile()
    return nc


def _host_prep(x, W, bias, adj, bs):
    """Build the per-core input maps."""
    diag = np.diagonal(adj).astype(np.float32)
    off = (adj * (1.0 - np.eye(J, dtype=adj.dtype))).astype(np.float32)

    # stage-1 weights: [FIN, J, 2*FOUT], columns = [diag_k*W0_k | W1_k]
    wcat = np.concatenate([diag[:, None, None] * W[0], W[1]], axis=2)
    wcat = np.ascontiguousarray(wcat.transpose(1, 0, 2)).astype(np.float32)

    # mixing stationary: rows r = i*34 + h*17 + k (h=0: h0s, h=1: h1),
    # rows 102+i: bias; cols (i'*17 + j)
    import ml_dtypes
    mixblock = np.zeros((34, J), dtype=np.float32)
    mixblock[0:J, :] = np.eye(J, dtype=np.float32)  # h0s rows
    mixblock[J:2 * J, :] = off.T      # h1 rows: sum_k off[j,k] h1_k
    mix3 = np.zeros((105, 51), dtype=np.float32)
    for i in range(3):
        mix3[i * 34:(i + 1) * 34, i * J:(i + 1) * J] = mixblock
        mix3[102 + i, i * J:(i + 1) * J] = 1.0      # bias row

    bias43 = np.tile(bias.astype(np.float32), (3, NG))
    ident = np.eye(128, dtype=np.float32)

    shared = {
        "wcat": wcat,
        "mix3": mix3.astype(ml_dtypes.bfloat16),
        "bias43": np.ascontiguousarray(bias43).astype(ml_dtypes.bfloat16),
        "ident": ident,
    }
    in_maps = []
    for c in range(N_CORES):
        m = dict(shared)
        m["xs"] = np.ascontiguousarray(x[c * bs:(c + 1) * bs])
        in_maps.append(m)
    return in_maps


def _run(x, W, bias, adj, bs, profile=False, tmpdir=None):
    if bs not in _prog_cache:
        _prog_cache[bs] = _build_program(bs)
    nc = _prog_cache[bs]
    in_maps = _host_prep(x, W, bias, adj, bs)
    res = run_bass_kernel_spmd(nc, in_maps, list(range(N_CORES)),
                               trace=profile, tmpdir=tmpdir)
    out = np.concatenate([res.results[c]["out"] for c in range(N_CORES)],
                         axis=0)
    if profile:
        return out, res
    return out


def kernel(x, W, bias, adj):
    x = np.asarray(x, dtype=np.float32)
    W = np.asarray(W, dtype=np.float32)
    bias = np.asarray(bias, dtype=np.float32)
    adj = np.asarray(adj, dtype=np.float32)
    assert x.shape == (B, J, FIN)
    return _run(x, W, bias, adj, B // N_CORES)


# revision 18
# speedup vs baseline: 1.6963x; 1.6963x over previous
"""Trainium2 Bass kernel for DecouplePreAggGraphConv (GNN message passing).

out[b,j,:] = diag(adj)[j] * (x[b,j] @ W0[j])
           + sum_k offdiag(adj)[j,k] * (x[b,k] @ W1[k])
           + bias

Data-parallel over B across 8 NeuronCores. Per core, per 128-row batch
tile:
  1. one DMA load of x-tile [128, J*128]
  2. PE transposes per joint -> xT_k [n, b] (via identity matmul)
  3. per-joint GEMM  h_k = xT_k.T @ [diag_k*W0_k | W1_k]  -> PSUM [128,256]
  4. drain h to SBUF, then SBUF->SBUF DMA reshuffle into a
     (3-batch-row-group, 35-row) layout: rows = [17 h1 | 17 h0s | bias]
  5. mixing GEMM with a constant block-diagonal [105,51] stationary
     matrix (off.T / I / ones blocks) computes the adjacency mix, the
     self term and the bias add in one pass -> PSUM [51, (g,m)]
  6. drain + one strided store straight into out[b,j,m] layout.
"""

import sys

sys.path.insert(0, "/opt/trn_rl_repo")

import numpy as np

import concourse.bass as bass
import concourse.mybir as mybir
import concourse.tile as tile
from concourse import bacc
from concourse.bass_utils import run_bass_kernel_spmd

B, J, FIN, FOUT = 16384, 17, 128, 128
N_CORES = 8
TB = 128            # batch rows per tile
CJ = J * FOUT       # 2176
G3 = TB // 3        # 42 full groups of 3 rows; rows 126/127 ride as group 42
MAIN = 3 * G3       # 126
NG = G3 + 1         # 43 group slots (last one only has i=0,1 valid)
HPF = NG * FOUT     # 5504 free size of the reshuffled tile
MIXCH = 1024        # mix psum chunk (free elems)
F32 = mybir.dt.float32
BF16 = mybir.dt.bfloat16

_prog_cache: dict[int, object] = {}


def _build_program(bs: int, repeat: int = 1):
    """Build the SPMD Bass program for a per-core batch shard of `bs` rows."""
    nt = bs // TB
    assert bs % TB == 0

    nc = bacc.Bacc("TRN2", target_bir_lowering=False, debug=False,
                   num_devices=N_CORES)

    xs = nc.declare_dram_parameter("xs", [bs, J, FIN], F32, isOutput=False)
    wcat = nc.declare_dram_parameter("wcat", [FIN, J, 2 * FOUT], F32,
                                     isOutput=False)
    mix3 = nc.declare_dram_parameter("mix3", [105, 51], BF16, isOutput=False)
    bias43 = nc.declare_dram_parameter("bias43", [3, HPF], BF16,
                                       isOutput=False)
    ident = nc.declare_dram_parameter("ident", [128, 128], F32, isOutput=False)
    out = nc.declare_dram_parameter("out", [bs, J, FOUT], F32, isOutput=True)

    with tile.TileContext(nc) as tc:
        with (
            tc.tile_pool(name="const", bufs=1) as cpool,
            tc.tile_pool(name="x", bufs=2) as xpool,
            tc.tile_pool(name="xt", bufs=3) as xtpool,
            tc.tile_pool(name="hsb", bufs=2) as hpool,
            tc.tile_pool(name="hp", bufs=2) as hppool,
            tc.tile_pool(name="osb", bufs=2) as opool,
            tc.tile_pool(name="tp", bufs=2, space=bass.MemorySpace.PSUM) as tpp,
            tc.tile_pool(name="hps", bufs=2, space=bass.MemorySpace.PSUM) as hpsp,
            tc.tile_pool(name="mxp", bufs=2, space=bass.MemorySpace.PSUM) as mxpp,
        ):
            # ---- constants, loaded once ----
            wcat_sb = cpool.tile([FIN, J, 2 * FOUT], F32, tag="wcat")
            nc.sync.dma_start(wcat_sb[:], wcat[:])
            mix3_sb = cpool.tile([105, 51], BF16, tag="mix3")
            nc.sync.dma_start(mix3_sb[:], mix3[:])
            id_sb = cpool.tile([128, 128], F32, tag="ident")
            nc.sync.dma_start(id_sb[:], ident[:])

            # ping-pong DRAM scratch for the reshuffle bounce; the
            # (i=2, g=42) rectangle never gets scattered into, so zero it
            # once (PE accumulates 0*garbage = NaN otherwise).
            scrs = [nc.dram_tensor(f"scr{p}", [102, HPF], BF16)
                    for p in range(2)]
            zro = cpool.tile([34, FOUT], BF16, tag="zro")
            nc.gpsimd.memset(zro[:], 0.0)
            for p in range(2):
                nc.sync.dma_start(
                    scrs[p][68:102, G3 * FOUT:], zro[:])

            for t in range(nt * repeat):
                t = t % nt
                b0 = t * TB
                # 1. load x tile
                x_t = xpool.tile([TB, J, FIN], F32, tag="x")
                nc.sync.dma_start(x_t[:], xs[b0:b0 + TB])

                # 2/3/4a. per joint: transpose, GEMM, drain (cast bf16)
                h_sb = hpool.tile([TB, 2, J, FOUT], BF16, tag="h")
                for k in range(J):
                    tp = tpp.tile([128, TB], F32, tag="tp")
                    nc.tensor.transpose(tp[:], x_t[:, k, :], id_sb[:])
                    xt = xtpool.tile([128, TB], F32, tag="xt")
                    if k % 2 == 0:
                        nc.vector.tensor_copy(xt[:], tp[:])
                    else:
                        nc.scalar.copy(xt[:], tp[:])
                    hk = hpsp.tile([TB, 2 * FOUT], F32, tag="hk")
                    nc.tensor.matmul(hk[:], xt[:], wcat_sb[:, k, :])
                    if k % 2 == 0:
                        nc.scalar.copy(h_sb[:, :, k, :], hk[:])
                    else:
                        nc.vector.tensor_copy(h_sb[:, :, k, :], hk[:])

                # 4b. reshuffle via DRAM bounce: scatter h into the group
                # layout in a DRAM scratch (rows r = i*34 + h*17 + k), then
                # read it back contiguously. DRAM APs have no partition-dim
                # restriction, so this is 3 scatter DMAs + 2 readback DMAs.
                scr = scrs[t % 2]
                sv = scr.rearrange("(i h k) (g m) -> i g h k m",
                                   i=3, h=2, k=17, g=NG, m=FOUT)
                for i in range(3):
                    ng = NG if i < 2 else G3
                    nc.sync.dma_start(sv[i, :ng], h_sb[i:TB:3])
                hp_t = hppool.tile([105, HPF], BF16, tag="hp")
                nc.sync.dma_start(hp_t[0:102, :], scr[:])
                nc.sync.dma_start(hp_t[102:105, :], bias43[:])

                # 5/6. mix GEMM chunks, drain, store
                # out[(i,j),(g,m)] = h0s[3g+i,j,m]
                #                  + sum_k off[j,k]*h1[3g+i,k,m] + bias[m]
                # (i=2, g=42) columns are garbage and never stored.
                o_sb = opool.tile([51, HPF], F32, tag="osb")
                nch = (HPF + MIXCH - 1) // MIXCH
                for c in range(nch):
                    f0 = c * MIXCH
                    fw = min(MIXCH, HPF - f0)
                    mp = mxpp.tile([51, MIXCH], F32, tag="mx")
                    for s0 in range(0, fw, 512):
                        sw = min(512, fw - s0)
                        nc.tensor.matmul(mp[:, s0:s0 + sw], mix3_sb[:],
                                         hp_t[:, f0 + s0:f0 + s0 + sw])
                    if c % 2 == 0:
                        nc.vector.tensor_copy(o_sb[:, f0:f0 + fw], mp[:, :fw])
                    else:
                        nc.scalar.copy(o_sb[:, f0:f0 + fw], mp[:, :fw])

                dst = out[b0:b0 + MAIN].rearrange("(g i) j m -> i j g m", i=3)
                nc.sync.dma_start(dst, o_sb[:, :G3 * FOUT])
                nc.sync.dma_start(out[b0 + MAIN:b0 + TB],
                                  o_sb[0:34, G3 * FOUT:])

    nc.compile()
    return nc


def _host_prep(x, W, bias, adj, bs):
    """Build the per-core input maps."""
    diag = np.diagonal(adj).astype(np.float32)
    off = (adj * (1.0 - np.eye(J, dtype=adj.dtype))).astype(np.float32)

    # stage-1 weights: [FIN, J, 2*FOUT], columns = [diag_k*W0_k | W1_k]
    wcat = np.concatenate([diag[:, None, None] * W[0], W[1]], axis=2)
    wcat = np.ascontiguousarray(wcat.transpose(1, 0, 2)).astype(np.float32)

    # mixing stationary: rows r = i*34 + h*17 + k (h=0: h0s, h=1: h1),
    # rows 102+i: bias; cols (i'*17 + j)
    import ml_dtypes
    mixblock = np.zeros((34, J), dtype=np.float32)
    mixblock[0:J, :] = np.eye(J, dtype=np.float32)  # h0s rows
    mixblock[J:2 * J, :] = off.T      # h1 rows: sum_k off[j,k] h1_k
    mix3 = np.zeros((105, 51), dtype=np.float32)
    for i in range(3):
        mix3[i * 34:(i + 1) * 34, i * J:(i + 1) * J] = mixblock
        mix3[102 + i, i * J:(i + 1) * J] = 1.0      # bias row

    bias43 = np.tile(bias.astype(np.float32), (3, NG))
    ident = np.eye(128, dtype=np.float32)

    shared = {
        "wcat": wcat,
        "mix3": mix3.astype(ml_dtypes.bfloat16),
        "bias43": np.ascontiguousarray(bias43).astype(ml_dtypes.bfloat16),
        "ident": ident,
    }
    in_maps = []
    for c in range(N_CORES):
        m = dict(shared)
        m["xs"] = np.ascontiguousarray(x[c * bs:(c + 1) * bs])
        in_maps.append(m)
    return in_maps


def _run(x, W, bias, adj, bs, profile=False, tmpdir=None):
    if bs not in _prog_cache:
        _prog_cache[bs] = _build_program(bs)
    nc = _prog_cache[bs]
    in_maps = _host_prep(x, W, bias, adj, bs)
    res = run_bass_kernel_spmd(nc, in_maps, list(range(N_CORES)),
                               trace=profile, tmpdir=tmpdir)
    out = np.concatenate([res.results[c]["out"] for c in range(N_CORES)],
                         axis=0)
    if profile:
        return out, res
    return out


def kernel(x, W, bias, adj):
    x = np.asarray(x, dtype=np.float32)
    W = np.asarray(W, dtype=np.float32)
    bias = np.asarray(bias, dtype=np.float32)
    adj = np.asarray(adj, dtype=np.float32)
    assert x.shape == (B, J, FIN)
    return _run(x, W, bias, adj, B // N_CORES)
